# revision 25
# baseline (speedup 1.0000x reference)
"""Trainium2 Bass kernel for nn_BettingLoss.

Strategy: pure data-parallel over B=1048576 across 8 NeuronCores (131072
rows/core). All inputs are converted to bf16 on the host with constant
affine prescales folded into the cast (o' = 2.09*odds, g' = 10*g - 70, p,
w) and packed into ONE DRAM tensor [P, NCH, 4, T, RC] (T-major within each
chunk), so each chunk is a single contiguous DMA.

HW-calibrated op economics (measured via loop-differencing probes; the
CoreSim cost model is wrong on this silicon): DVE tensor-tensor ADD and
all tensor_scalar forms hit a fast SIMD path (~0.15-0.25 ns/elem), DVE
tensor-tensor MULT runs ~1.0 ns/elem, Pool TT ~1.7, ACT ~1.4 (bf16 IO),
InstReciprocal ~6.2, accum_out on DVE adds ~2.3us per op. The kernel
therefore: does T-group reductions as packed TT-ADD fold chains over one
6-slot pack; computes 1/o' and ln(p) with int16 bit tricks on the
tensor_scalar fast path; keeps the two exps on ACT; splits the four
unavoidable tensor multiplies DVE(aa,wp)/Pool(t2,ej); and uses no
accumulators at all (global sums = one TensorReduce per tail half).

Per chunk c (software-pipelined so every engine queue stays dependency
free): DMA(c) -> DVE aa=o'*p, wp=w*p, zz=aa+g', rcp=magic(o'),
IP=bits(p)-16256 -> ACT pe=exp(p), e=exp(zz) -> Pool t2=aa*e, IPp=p*IP
[stage c-1] -> DVE 3 fold adds over [e,t2,pe,rcp,wp,IPp] -> SMB[c]
[stage c-2]. Tail (per half): validf=simp>=TH, r=1/es via magic+Newton,
tsr=ts*r, ce=ln(pes)-wps, masked sums via one [P,4,..]->[P,4] reduce.
Host sums partitions in f64 and applies the final scalar formula
(lambda_betting saturates at 1; ln(p) bit-trick bias corrected by the
analytic uniform-mantissa constant 2ln2-1-ln2/2).
"""

import numpy as np
import ml_dtypes

import concourse.bacc as bacc
import concourse.tile as tile
from concourse import mybir
from concourse.bass_utils import run_bass_kernel_spmd

N_CORES = 8
B, T = 1048576, 8
BSH = B // N_CORES          # 131072 rows per core
P = 128                     # SBUF partitions
ROWS_PP = BSH // P          # 1024 rows per partition
NCH = 8                     # chunks along the free dim
RC = ROWS_PP // NCH         # 128 rows per partition per chunk
NH = NCH // 2               # chunks per tail half

F32 = mybir.dt.float32
BF16 = mybir.dt.bfloat16
FP16 = mybir.dt.float16
I16 = mybir.dt.int16
ALU = mybir.AluOpType
AFT = mybir.ActivationFunctionType

EXP_SHIFT = 70.0            # folded into host g' = 10*g - EXP_SHIFT
TH = 0.95 / 2.09            # validity threshold in o'=2.09*o space
MAGIC_K = 0x7EF1            # bf16 reciprocal magic constant
B_LN = 0x3F80               # bf16 bits of 1.0 (exponent reference for ln)
S_LN = float(np.log(2.0) / 128.0)   # ln(x) ~ S_LN*(bits(x)-B_LN)
C_LN = float(2 * np.log(2.0) - 1.0 - np.log(2.0) / 2.0)  # E[ln(1+m)-m*ln2]

last_exec_time_ns = None
last_results = None

_BUILT = {}


def _patch_act_tables():
    """Steer the act-table-load pass to the one set that has BOTH Exp and Ln
    (natural_log_exp_and_others) so the kernel pays a single table load."""
    if getattr(bacc, "_act_tables_patched", False):
        return
    orig = bacc.get_activation_tables

    def patched(arch):
        tables = {k: set(v) for k, v in orig(arch).items()}
        AFT_ = mybir.ActivationFunctionType
        for name, funcs in tables.items():
            if name != "natural_log_exp_and_others":
                funcs.discard(AFT_.Exp)
                funcs.discard(AFT_.Ln)
        return tables

    bacc.get_activation_tables = patched
    bacc._act_tables_patched = True


def _emit_chunks(nc, tc, pools, out_t, allin_d):
    pin, ppk, pmid, psm = pools

    # persistent fold results: [P, NCH, 6, RC]
    # slot order: 0=es 1=ts 2=pes 3=simp 4=wps 5=ents
    SMB = psm.tile([P, NCH, 6, RC], BF16, tag="smb", name="smb")

    INs, PKs, IPs = {}, {}, {}

    def stage_front(c):
        # DMA slots: 0=w 1=o' 2=p 3=g'
        IN = pin.tile([P, 4, T, RC], BF16, tag="in", name=f"in{c}")
        nc.sync.dma_start(out=IN, in_=allin_d[:, c])
        INs[c] = IN
        w_, o_, p_, g_ = IN[:, 0], IN[:, 1], IN[:, 2], IN[:, 3]

        # fold pack: 0=e 1=t2 2=pe 3=rcp 4=wp 5=IPp (6=aa, not folded)
        PK = ppk.tile([P, 7, T, RC], BF16, tag="pk", name=f"pk{c}")
        PKs[c] = PK
        aa = PK[:, 6]

        # DVE multiplies (slow class, ~1 ns/elem) and fast TS bit tricks
        nc.vector.tensor_tensor(out=aa, in0=o_, in1=p_, op=ALU.mult)
        nc.vector.tensor_tensor(out=PK[:, 4], in0=w_, in1=p_, op=ALU.mult)
        zz = pmid.tile([P, T, RC], BF16, tag="zz", name=f"zz{c}")
        nc.vector.tensor_tensor(out=zz, in0=aa, in1=g_, op=ALU.add)
        # rcp = 1/o' via magic bits (feeds only the simp>=TH test)
        nc.vector.tensor_scalar(out=PK[:, 3].bitcast(I16),
                                in0=o_.bitcast(I16),
                                scalar1=float(MAGIC_K), scalar2=-1.0,
                                op0=ALU.subtract, op1=ALU.mult)
        # IP = bits(p) - B_LN, exact in fp16 (|I-B| < 2048): ln(p) ~ S_LN*IP
        IP = pmid.tile([P, T, RC], FP16, tag="ip", name=f"ip{c}")
        nc.vector.tensor_scalar(out=IP, in0=p_.bitcast(I16),
                                scalar1=float(B_LN), scalar2=0.0,
                                op0=ALU.subtract, op1=ALU.add)
        IPs[c] = IP

        # ACT: pe first (DMA-dep only, runs while zz lands), then e
        nc.scalar.activation(out=PK[:, 2], in_=p_, func=AFT.Exp)
        nc.scalar.activation(out=PK[:, 0], in_=zz, func=AFT.Exp)

    def stage_mid(c):
        PK = PKs[c]
        # Pool multiplies (deps finished one stage ago)
        nc.gpsimd.tensor_tensor(out=PK[:, 1], in0=PK[:, 6], in1=PK[:, 0],
                                op=ALU.mult)
        nc.gpsimd.tensor_tensor(out=PK[:, 5], in0=INs.pop(c)[:, 2],
                                in1=IPs.pop(c), op=ALU.mult)

    def stage_folds(c):
        # DVE fold chain (TT adds, fast path; T-major keeps all three levels
        # stride-1 in the innermost dim)
        PK = PKs.pop(c)
        F1 = pmid.tile([P, 6, 4, RC], BF16, tag="f1", name=f"f1{c}")
        nc.vector.tensor_tensor(out=F1, in0=PK[:, 0:6, 0:4],
                                in1=PK[:, 0:6, 4:8], op=ALU.add)
        F2 = pmid.tile([P, 6, 2, RC], BF16, tag="f2", name=f"f2{c}")
        nc.vector.tensor_tensor(out=F2, in0=F1[:, :, 0:2], in1=F1[:, :, 2:4],
                                op=ALU.add)
        nc.vector.tensor_tensor(out=SMB[:, c], in0=F2[:, :, 0],
                                in1=F2[:, :, 1], op=ALU.add)

    def tail_half(h):
        # batched per-row tail over chunks [h*NH, (h+1)*NH)
        sl = slice(h * NH, (h + 1) * NH)
        FH = NH * RC
        es_b = SMB[:, sl, 0]
        ts_b = SMB[:, sl, 1]
        pes_b = SMB[:, sl, 2]
        simp_b = SMB[:, sl, 3]
        wps_b = SMB[:, sl, 4]
        ent_b = SMB[:, sl, 5]

        # masked-sum pack: 0=validf 1=tsr*vf 2=ce*vf 3=ent
        TP = psm.tile([P, 4, NH, RC], BF16, tag=f"tp{h}", name=f"tp{h}")
        nc.vector.tensor_scalar(out=TP[:, 0], in0=simp_b, scalar1=TH,
                                scalar2=0.0, op0=ALU.is_ge, op1=ALU.add)
        # copy ent row-sums into the pack (fast TS path)
        nc.vector.tensor_scalar(out=TP[:, 3], in0=ent_b, scalar1=1.0,
                                scalar2=0.0, op0=ALU.mult, op1=ALU.add)

        # r = 1/es: magic seed + one Newton step (es feeds values, not a
        # threshold, so the bare ~6% sawtooth is refined to ~0.4%)
        y0 = psm.tile([P, NH, RC], BF16, tag=f"y0{h}", name=f"y0{h}")
        nc.vector.tensor_scalar(out=y0.bitcast(I16), in0=es_b.bitcast(I16),
                                scalar1=float(MAGIC_K), scalar2=-1.0,
                                op0=ALU.subtract, op1=ALU.mult)
        u = psm.tile([P, NH, RC], BF16, tag=f"u{h}", name=f"u{h}")
        nc.vector.tensor_tensor(out=u, in0=es_b, in1=y0, op=ALU.mult)
        v = psm.tile([P, NH, RC], BF16, tag=f"v{h}", name=f"v{h}")
        nc.vector.tensor_scalar(out=v, in0=u, scalar1=-1.0, scalar2=2.0,
                                op0=ALU.mult, op1=ALU.add)
        r = psm.tile([P, NH, RC], BF16, tag=f"r{h}", name=f"r{h}")
        nc.vector.tensor_tensor(out=r, in0=y0, in1=v, op=ALU.mult)
        tsr = psm.tile([P, NH, RC], BF16, tag=f"tsr{h}", name=f"tsr{h}")
        nc.vector.tensor_tensor(out=tsr, in0=ts_b, in1=r, op=ALU.mult)
        nc.vector.tensor_tensor(out=TP[:, 1], in0=tsr, in1=TP[:, 0],
                                op=ALU.mult)

        lse = psm.tile([P, NH, RC], BF16, tag=f"lse{h}", name=f"lse{h}")
        nc.scalar.activation(out=lse, in_=pes_b, func=AFT.Ln)
        ce = psm.tile([P, NH, RC], BF16, tag=f"ce{h}", name=f"ce{h}")
        nc.vector.tensor_tensor(out=ce, in0=lse, in1=wps_b, op=ALU.subtract)
        nc.vector.tensor_tensor(out=TP[:, 2], in0=ce, in1=TP[:, 0],
                                op=ALU.mult)

        # one full-axis reduce for all four global sums of this half
        nc.vector.tensor_reduce(out=out_t[:, h * 4:(h + 1) * 4],
                                in_=TP.bitcast(BF16).rearrange(
                                    "p q n r -> p q (n r)"),
                                axis=mybir.AxisListType.X, op=ALU.add)

    for v in range(NCH + 2):
        if v < NCH:
            stage_front(v)
        if 1 <= v <= NCH:
            stage_mid(v - 1)
        if v >= 2:
            stage_folds(v - 2)
        if v == NCH // 2 + 1:
            tail_half(0)
    tail_half(1)


def _build(timing_iters=None):
    """timing_iters=None: grading build (ExternalInputs, single pass).
    timing_iters=R: benchmark build (Internal DRAM inputs, hardware For_i
    loop of R iterations; measure via wall-clock differencing)."""
    key = timing_iters
    if key in _BUILT:
        return _BUILT[key]

    _patch_act_tables()
    nc = bacc.Bacc("TRN2", target_bir_lowering=False, debug=False)
    kind = "ExternalInput" if timing_iters is None else "Internal"
    allin_d = nc.dram_tensor("allin", [P, NCH, 4, T, RC], BF16, kind=kind)
    if timing_iters is not None:
        dum_d = nc.dram_tensor("dum", [1, 4], F32, kind="ExternalInput")
    acc_d = nc.dram_tensor("acc", [P, 8], F32, kind="ExternalOutput")

    with tile.TileContext(nc) as tc:
        with (
            tc.tile_pool(name="pin", bufs=4) as pin,
            tc.tile_pool(name="ppk", bufs=4) as ppk,
            tc.tile_pool(name="pmid", bufs=3) as pmid,
            tc.tile_pool(name="psm", bufs=1) as psm,
            tc.tile_pool(name="pacc", bufs=1) as pacc,
        ):
            out_t = pacc.tile([P, 8], F32, tag="out", name="out")
            pools = (pin, ppk, pmid, psm)
            with nc.allow_low_precision(reason="bf16 kernel; 2e-2 tolerance"):
                if timing_iters is None:
                    _emit_chunks(nc, tc, pools, out_t, allin_d)
                else:
                    dumt = pacc.tile([1, 4], F32, tag="dum", name="dumt")
                    nc.sync.dma_start(out=dumt, in_=dum_d[:])
                    with tc.For_i(0, timing_iters, 1):
                        for _ in range(TIMING_INNER):
                            _emit_chunks(nc, tc, pools, out_t, allin_d)
            nc.sync.dma_start(out=acc_d[:], in_=out_t)

    nc.compile()
    _BUILT[key] = nc
    return nc


TIMING_INNER = 2


def _run_timing(iters, reps=3):
    import time
    nc = _build(timing_iters=iters)
    in_maps = [{"dum": np.zeros((1, 4), np.float32)} for _ in range(N_CORES)]
    best = None
    for _ in range(reps):
        t0 = time.time()
        run_bass_kernel_spmd(nc, in_maps, list(range(N_CORES)))
        dt = time.time() - t0
        best = dt if best is None else min(best, dt)
    return best


def measure_hw_ns(lo=100, hi=1600, reps=4, trials=3):
    """HW ns per kernel invocation via loop-count differencing."""
    _run_timing(lo, reps=1)  # warm compile+cache
    _run_timing(hi, reps=1)
    ests = []
    for _ in range(trials):
        tlo = _run_timing(lo, reps=reps)
        thi = _run_timing(hi, reps=reps)
        ests.append((thi - tlo) / (hi - lo) / TIMING_INNER * 1e9)
    return float(np.median(ests))


def _prep(predicted_probs, true_winners, market_odds, gumbel_noise):
    """Host-side shard + prescale + bf16 cast + T-major pack."""
    bf16 = ml_dtypes.bfloat16

    def tmaj(a):
        # [BSH, T] f32 -> [P, NCH, T, RC] bf16 (T-major within chunk)
        return np.ascontiguousarray(
            a.reshape(P, NCH, RC, T).transpose(0, 1, 3, 2))

    in_maps = []
    for k in range(N_CORES):
        s = slice(k * BSH, (k + 1) * BSH)
        w = tmaj(true_winners[s].astype(bf16))
        o = tmaj((market_odds[s] * np.float32(2.09)).astype(bf16))
        p = tmaj(predicted_probs[s].astype(bf16))
        g = tmaj((gumbel_noise[s] * np.float32(10.0)
                  - np.float32(EXP_SHIFT)).astype(bf16))
        allin = np.ascontiguousarray(
            np.stack([w, o, p, g], axis=2))  # [P, NCH, 4, T, RC]
        in_maps.append({"allin": allin})
    return in_maps


def kernel(predicted_probs, true_winners, market_odds, gumbel_noise):
    global last_exec_time_ns, last_results
    nc = _build()
    in_maps = _prep(predicted_probs, true_winners, market_odds, gumbel_noise)
    res = run_bass_kernel_spmd(nc, in_maps, list(range(N_CORES)))
    last_results = res

    S = np.zeros(8, dtype=np.float64)
    for k in range(N_CORES):
        S += res.results[k]["acc"].astype(np.float64).sum(axis=0)

    # halves: [cnt, q4, cev, ent] each
    cnt = S[0] + S[4]
    q4S = S[1] + S[5]
    cevS = S[2] + S[6]
    entS = S[3] + S[7]

    # soft_ep per valid row = tsr/100 - 0.019 (tsr in aa=2.09*o*p space)
    pred = cevS / max(cnt, 1.0)
    bet = -(q4S / 100.0 - 0.019 * cnt) / B
    # ln(p) ~ S_LN*(bits(p)-B_LN) + C_LN (uniform-mantissa mean correction);
    # sum of p over each row is 1, so the correction term is C_LN*B exactly
    ent_sum = S_LN * entS + C_LN * B
    entreg = -ent_sum / B
    lam = min(0.5 + cnt / 10000.0 * 0.5, 1.0)
    loss = pred + lam * bet - 0.01 * entreg
    return np.array(loss, dtype=np.float32)
